# revision 18
# baseline (speedup 1.0000x reference)
"""TRN2 Bass/Tile kernel: deformable-kernel spatial attention (dense_cnn).

Per-core (pure data parallel, batch 8 over 8 cores):
  x cached in SBUF as fp16 with column-parity-split layout (single HBM
  read, contiguous moving-operand reads for the stride-2 conv1 and the
  final attention multiply). Spatial halves packed on partitions 0:64 /
  64:128 so every matmul runs K=128 with block-diagonal weights.
  h1 = relu(conv1(x))     3x3 stride-2 64->64, 9 tap-matmuls, 4-row bands
  5x dkc:                 global-pool -> fc offsets -> hat-function
                          resample of the 4x4 scope kernel -> depthwise
                          3x3: 7 diagonal matmuls on PE + 2 DVE FMAs
  conv2+pixel_shuffle+conv3 fused into a 64->4 channel 3x3 conv
  att replicated across channels with K=8 selector matmuls; final
  out = x * att on DVE from the fp16 x cache.

Queue discipline: sync = x top-half loads + out top-half stores;
gpsimd = x bottom-half loads + out bottom-half stores + replicates;
tensor queue issues its own weight-staging DMAs and halo copies so the
PE stream is gated naturally; V/S do the fp32->fp16 casts for the
top/bottom halves respectively.
"""

import numpy as np

import concourse.bass as bass
import concourse.mybir as mybir
import concourse.tile as tile
from concourse import bacc
from concourse.bass_utils import run_bass_kernel_spmd
from concourse.masks import make_identity
from contextlib import ExitStack

f32 = mybir.dt.float32
f32r = mybir.dt.float32r
f16 = mybir.dt.float16
AF = mybir.ActivationFunctionType
ALU = mybir.AluOpType
AX = mybir.AxisListType

C = 64
H = 256
HH = 128
RB = 64          # interior feature rows per half
SLOTS = RB + 2   # + top/bottom halo row
WCOL = HH + 2    # zero gutter columns at 0 and 129
XSL = 130        # x-cache slots per half: halo, 128 rows, (unused)
XW = 257         # x-cache cols: [zero | odd cols 1..128 | even cols 129..256]
NL = 5
BR = 4           # output rows per band (moving operand max 512 elements)
NB = RB // BR    # bands per conv phase
XCH = 8          # x rows per load chunk (per half)
NCH = 128 // XCH

TAPS = [(t // 3 - 1, t % 3 - 1) for t in range(9)]  # t = 3*ty+tx -> (dy, dx)
PE_TAPS = [0, 1, 2, 3, 4, 6, 8]
DVE_TAPS = [5, 7]
# x-cache column base for a tap reading input col 2*xo+dx
XCB = {-1: 0, 0: 129, 1: 1}


def _ap(a, extra_off, dims):
    return bass.AP(tensor=a.tensor, offset=a.offset + extra_off, ap=dims)


def build_nc():
    nc = bacc.Bacc("TRN2", target_bir_lowering=False, debug=False)
    x_d = nc.dram_tensor("x", [C, H, H], f32, kind="ExternalInput").ap()
    w1_d = nc.dram_tensor("conv1_w", [C, C, 3, 3], f32, kind="ExternalInput").ap()
    b1_d = nc.dram_tensor("conv1_b", [C], f32, kind="ExternalInput").ap()
    dkw_d = nc.dram_tensor("dkc_w", [NL, C, 1, 4, 4], f32, kind="ExternalInput").ap()
    dkb_d = nc.dram_tensor("dkc_b", [NL, C], f32, kind="ExternalInput").ap()
    fcw_d = nc.dram_tensor("dkc_fc_w", [NL, 18, C], f32, kind="ExternalInput").ap()
    fcb_d = nc.dram_tensor("dkc_fc_b", [NL, 18], f32, kind="ExternalInput").ap()
    w2_d = nc.dram_tensor("conv2_w", [4 * C, C, 3, 3], f32, kind="ExternalInput").ap()
    b2_d = nc.dram_tensor("conv2_b", [4 * C], f32, kind="ExternalInput").ap()
    w3_d = nc.dram_tensor("conv3_w", [1, C, 1, 1], f32, kind="ExternalInput").ap()
    b3_d = nc.dram_tensor("conv3_b", [1], f32, kind="ExternalInput").ap()
    o_d = nc.dram_tensor("out", [C, H, H], f32, kind="ExternalOutput").ap()

    with tile.TileContext(nc) as tc:
        with ExitStack() as ctx:
            _kernel(ctx, tc, nc, x_d, w1_d, b1_d, dkw_d, dkb_d, fcw_d, fcb_d,
                    w2_d, b2_d, w3_d, b3_d, o_d)
    nc.compile()
    return nc


def _kernel(ctx, tc, nc, x_d, w1_d, b1_d, dkw_d, dkb_d, fcw_d, fcb_d,
            w2_d, b2_d, w3_d, b3_d, o_d):
    persist = ctx.enter_context(tc.tile_pool(name="persist", bufs=1))
    xcp = ctx.enter_context(tc.tile_pool(name="xcp", bufs=1))
    hpool = ctx.enter_context(tc.tile_pool(name="h", bufs=2))
    stagep = ctx.enter_context(tc.tile_pool(name="stage", bufs=4))
    small = ctx.enter_context(tc.tile_pool(name="small", bufs=4))
    diagp = ctx.enter_context(tc.tile_pool(name="diag", bufs=2))
    outp = ctx.enter_context(tc.tile_pool(name="outb", bufs=3))
    psum = ctx.enter_context(tc.tile_pool(name="psum", bufs=6, space="PSUM"))
    psmall = ctx.enter_context(tc.tile_pool(name="psmall", bufs=2, space="PSUM"))

    # ---------------- early memsets (V) ----------------
    zrow = persist.tile([128, XW], f32)
    nc.vector.memset(zrow[:], 0.0)
    w1t = persist.tile([128, 9, 128], f16)
    nc.vector.memset(w1t[:], 0.0)
    w2ft = persist.tile([128, 9, 8], f16)
    nc.vector.memset(w2ft[:], 0.0)

    # ---------------- gpsimd: tiny iota/identity/selector setup ----------------
    i128 = persist.tile([128, 128], f32)
    make_identity(nc, i128[:])
    it = small.tile([16, 18], mybir.dt.int32, tag="it")
    nc.gpsimd.iota(it[:, 0:9], pattern=[[1, 3], [0, 3]], base=0, channel_multiplier=0)
    nc.gpsimd.iota(it[:, 9:18], pattern=[[0, 3], [1, 3]], base=0, channel_multiplier=0)
    itp = small.tile([16, 1], mybir.dt.int32, tag="itp")
    nc.gpsimd.iota(itp[:], pattern=[[0, 1]], base=0, channel_multiplier=1)
    st16 = small.tile([1, 16], mybir.dt.int32, tag="st16")
    nc.gpsimd.iota(st16[:], pattern=[[1, 4], [0, 4]], base=0, channel_multiplier=0)
    sel8f = persist.tile([8, 4, 128], f32)
    nc.gpsimd.memset(sel8f[:], 0.0)
    for j in range(4):
        for g in range(2):
            nc.gpsimd.affine_select(out=sel8f[:, j, 64 * g:64 * (g + 1)],
                                    in_=sel8f[:, j, 64 * g:64 * (g + 1)],
                                    pattern=[[0, 64]], compare_op=ALU.not_equal,
                                    fill=1.0, base=-(4 * g + j),
                                    channel_multiplier=1)

    # conv1 weights: one contiguous load, then PE transpose -> block-diag fp16
    w1flat = persist.tile([C, 576], f32)
    nc.scalar.dma_start(out=w1flat[:], in_=_ap(w1_d, 0, [[576, C], [1, 576]]))
    c2wj = persist.tile([C, 4, 576], f32r)
    for j in range(4):
        nc.scalar.dma_start(out=c2wj[:, j, :],
                            in_=_ap(w2_d.bitcast(f32r), j * 576, [[4 * 576, C], [1, 576]]))
    w3sb = persist.tile([C, 4], f32r)
    nc.scalar.dma_start(out=w3sb[:].unsqueeze(-1),
                        in_=_ap(w3_d.bitcast(f32r), 0, [[1, C], [0, 4], [1, 1]]))
    c2bj = persist.tile([C, 4], f32r)
    nc.scalar.dma_start(out=c2bj[:], in_=_ap(b2_d.bitcast(f32r), 0, [[4, C], [1, 4]]))
    biases = persist.tile([128, 6], f32)
    nc.scalar.dma_start(out=biases[0:C, 0:1], in_=b1_d.unsqueeze(-1))
    for i in range(NL):
        nc.scalar.dma_start(out=biases[0:C, 1 + i:2 + i], in_=dkb_d[i].unsqueeze(-1))
    w2dt2 = persist.tile([16, NL, 128], f32r)
    for i in range(NL):
        dsrc = _ap(dkw_d.bitcast(f32r), i * 1024, [[1, 16], [16, C]])
        nc.scalar.dma_start(out=w2dt2[:, i, 0:C], in_=dsrc)
        nc.scalar.dma_start(out=w2dt2[:, i, C:128], in_=dsrc)
    fcwtf = persist.tile([128, NL, 18], f32)
    for i in range(NL):
        fsrc = _ap(fcw_d, i * 18 * C, [[1, C], [C, 18]])
        nc.scalar.dma_start(out=fcwtf[0:C, i, :], in_=fsrc)
        nc.scalar.dma_start(out=fcwtf[C:128, i, :], in_=fsrc)
    fcb16 = persist.tile([16, NL, 18], f32)
    nc.scalar.dma_start(out=fcb16[:], in_=_ap(fcb_d, 0, [[0, 16], [18, NL], [1, 18]]))
    b3b8 = small.tile([8, 1], f32, tag="b3b")
    nc.scalar.dma_start(out=b3b8[:], in_=_ap(b3_d, 0, [[0, 8], [1, 1]]))
    for g3 in range(3):
        tp = psum.tile([128, BR, HH], f32, tag="cps")
        for tt in range(3):
            t = 3 * g3 + tt
            in_t = _ap(w1flat, t, [w1flat.ap[0], [9, C]])
            nc.tensor.transpose(tp[0:C, tt, 0:C], in_t, i128[0:C, 0:C])
        nc.scalar.activation(w1t[0:C, 3 * g3:3 * g3 + 3, 0:C], tp[0:C, 0:3, 0:C],
                             AF.Copy, bias=0.0, scale=1.0)
    nc.scalar.dma_start(out=w1t[C:128, :, C:128], in_=w1t[0:C, :, 0:C])

    # ---------------- x cache gutters (S) ----------------
    xc = xcp.tile([128, XSL, XW], f16)
    zc = _ap(zrow, 0, [zrow.ap[0], [1, XSL], [1, 1]])
    nc.scalar.activation(xc[:, :, 0:1], zc, AF.Copy, bias=0.0, scale=1.0)
    nc.scalar.activation(xc[0:C, 0, :], zrow[0:C, 0:XW], AF.Copy, bias=0.0, scale=1.0)
    nc.scalar.activation(xc[:, XSL - 1, :], zrow[:, 0:XW], AF.Copy, bias=0.0, scale=1.0)

    # ---------------- x load + cast (chunk 15 first for the halo) ----------------
    for ci, k in enumerate([NCH - 1] + list(range(NCH - 1))):
        r0 = XCH * k
        st = stagep.tile([128, XCH, H], f32, tag="xst")
        nc.sync.dma_start(out=st[0:C], in_=x_d[:, r0:r0 + XCH, :])
        nc.gpsimd.dma_start(out=st[C:128], in_=x_d[:, 128 + r0:128 + r0 + XCH, :])
        # casts (odd cols -> 1..128, even -> 129..256): V does 3, S does 1
        nc.vector.tensor_copy(xc[0:C, 1 + r0:1 + r0 + XCH, 1:129], st[0:C, :, 1:H:2])
        nc.vector.tensor_copy(xc[0:C, 1 + r0:1 + r0 + XCH, 129:257], st[0:C, :, 0:H:2])
        nc.vector.tensor_copy(xc[C:128, 1 + r0:1 + r0 + XCH, 1:129], st[C:128, :, 1:H:2])
        nc.scalar.activation(xc[C:128, 1 + r0:1 + r0 + XCH, 129:257], st[C:128, :, 0:H:2],
                             AF.Copy, bias=0.0, scale=1.0)
        if k == NCH - 1:
            nc.scalar.dma_start(out=xc[C:128, 0, :], in_=xc[0:C, 128, :])

    # ---------------- h tensors ----------------
    def new_h():
        h = hpool.tile([128, SLOTS, WCOL], f16, tag="h")
        nc.scalar.activation(h[0:C, 0, :], zrow[0:C, 0:WCOL], AF.Copy, bias=0.0, scale=1.0)
        nc.scalar.activation(h[C:128, SLOTS - 1, :], zrow[C:128, 0:WCOL], AF.Copy, bias=0.0, scale=1.0)
        zch = _ap(zrow, 0, [zrow.ap[0], [1, SLOTS], [1, 1]])
        nc.scalar.activation(h[:, :, 0:1], zch, AF.Copy, bias=0.0, scale=1.0)
        nc.scalar.activation(h[:, :, WCOL - 1:WCOL], zch, AF.Copy, bias=0.0, scale=1.0)
        return h

    def halo_fix(h):
        nc.scalar.dma_start(out=h[C:128, 0, :], in_=h[0:C, RB, :])
        nc.scalar.dma_start(out=h[0:C, SLOTS - 1, :], in_=h[C:128, 1, :])

    # ---------------- conv1 (stride 2) ----------------
    h1 = new_h()
    pp1 = small.tile([128, NB], f32, tag="pp")
    for b in range(NB):
        ps = psum.tile([128, BR, HH], f32, tag="cps")
        for t in range(9):
            dy, dx = TAPS[t]
            cb = XCB[dx]
            rhs = xc[:, 1 + 2 * BR * b + dy:1 + 2 * BR * b + dy + 2 * BR - 1:2,
                     cb:cb + HH]
            nc.tensor.matmul(ps[:], w1t[:, t, :], rhs,
                             start=(t == 0), stop=(t == 8))
        s0 = 1 + BR * b
        nc.scalar.activation(h1[:, s0:s0 + BR, 1:HH + 1], ps[:],
                             AF.Relu, bias=biases[:, 0:1], scale=1.0,
                             accum_out=pp1[:, b:b + 1])
    halo_fix(h1)

    # fused conv2/conv3 weight computation (PE, runs during the x load tail)
    w2f_ps = psmall.tile([C, 144], f32, tag="sp")
    for t in range(9):
        for j in range(4):
            lhsT = _ap(c2wj, j * 576 + t, [c2wj.ap[0], [9, C]])
            k = (t * 4 + j) * 4
            nc.tensor.matmul(w2f_ps[:, k:k + 4], lhsT, w3sb[:],
                             start=True, stop=True)
    b2f_ps = psmall.tile([4, 4], f32, tag="sp")
    nc.tensor.matmul(b2f_ps[:], c2bj[:], w3sb[:], start=True, stop=True)

    # ---------------- chain constants (emitted late; needed from ~dkc1) ----------------
    b18 = persist.tile([16, 18], f32)
    nc.vector.tensor_copy(b18[:], it[:])
    nc.vector.tensor_scalar_add(b18[:], b18[:], 0.5)
    idx16 = small.tile([16, 1], f32, tag="idx16")
    nc.vector.tensor_copy(idx16[:], itp[:])
    stf = small.tile([1, 16], f32, tag="stf")
    nc.vector.tensor_copy(stf[:], st16[:])
    ky16 = persist.tile([16, 1], f32)
    kx16 = persist.tile([16, 1], f32)
    nc.gpsimd.dma_start(out=ky16[:], in_=_ap(stf, 0, [[1, 16], [16, 1]]))
    nc.vector.scalar_tensor_tensor(kx16[:], ky16[:], -4.0, idx16[:], ALU.mult, ALU.add)
    ones1618 = persist.tile([16, 18], f32)
    nc.vector.memset(ones1618[:], 1.0)
    k18 = persist.tile([16, 18], f32)
    nc.vector.tensor_scalar(k18[:, 0:9], ones1618[:, 0:9], ky16[:], None, ALU.mult)
    nc.vector.tensor_scalar(k18[:, 9:18], ones1618[:, 9:18], kx16[:], None, ALU.mult)
    bk18 = persist.tile([16, NL, 18], f32)
    bmk = persist.tile([16, 18], f32)
    nc.vector.tensor_tensor(bmk[:], b18[:], k18[:], ALU.subtract)
    for i in range(NL):
        nc.vector.tensor_tensor(bk18[:, i, :], bmk[:], fcb16[:, i, :], ALU.add)
    kint = persist.tile([16, 10], f32r)
    nc.vector.tensor_copy(kint[:], zrow[0:16, 0:10])
    ones116f = persist.tile([1, 16], f32)
    nc.vector.memset(ones116f, 1.0)
    ones116 = persist.tile([1, 16], f32r)
    nc.vector.tensor_copy(ones116[:], ones116f[:])
    fcwt2 = persist.tile([128, NL, 18], f32r)
    nc.vector.tensor_scalar(fcwt2[:], fcwtf[:], 1.0 / 16384.0, None, ALU.mult)
    # replicates + small staging off the critical queues
    nc.gpsimd.dma_start(out=biases[C:128, :], in_=biases[0:C, :])
    w2fv = _ap(w2f_ps, 0, [w2f_ps.ap[0], [16, 9], [4, 4]])
    nc.scalar.activation(w2ft[0:C, :, 0:4], w2fv, AF.Copy, bias=0.0, scale=1.0)
    nc.gpsimd.dma_start(out=w2ft[C:128, :, 4:8], in_=w2ft[0:C, :, 0:4])
    b2f8 = persist.tile([8, 1], f32)
    nc.scalar.activation(b2f8[0:4], b2f_ps[:, 0:1], AF.Copy, bias=0.0, scale=1.0)
    nc.gpsimd.dma_start(out=b2f8[4:8], in_=b2f8[0:4])
    nc.vector.tensor_add(b2f8[:], b2f8[:], b3b8[:])
    sel8 = persist.tile([8, 4, 128], f16)
    nc.scalar.activation(sel8[:], sel8f[:], AF.Copy, bias=0.0, scale=1.0)
    att8 = persist.tile([8, RB, HH], f16)

    # ---------------- dkc layers ----------------
    h_cur, pp_cur = h1, pp1
    for li in range(NL):
        redr = small.tile([128, 1], f32r, tag="redr")
        with nc.allow_low_precision(reason="f32r is fp32 bits (PE fast-load fmt)"):
            nc.vector.tensor_reduce(out=redr[:], in_=pp_cur[:], axis=AX.X, op=ALU.add)
        offp = psmall.tile([1, 18], f32, tag="sp")
        nc.tensor.matmul(offp[:], redr[:], fcwt2[:, li, :], start=True, stop=True)
        off = small.tile([1, 18], f32r, tag="off")
        nc.vector.tensor_copy(off[:], offp[:])
        offr_ps = psmall.tile([16, 18], f32, tag="sp")
        nc.tensor.matmul(offr_ps[:], ones116[:], off[:], start=True, stop=True)
        # phi = relu(1 - |offp + b - k|) = relu(min(1 - d, 1 + d))
        phi = small.tile([16, 18], f32, tag="phi")
        phia = small.tile([16, 18], f32, tag="phia")
        nc.vector.tensor_tensor(phi[:], offr_ps[:], bk18[:, li, :], ALU.add)
        nc.vector.scalar_tensor_tensor(phia[:], phi[:], 1.0, ones1618[:],
                                       ALU.mult, ALU.add)
        nc.vector.scalar_tensor_tensor(phi[:], phi[:], -1.0, ones1618[:],
                                       ALU.mult, ALU.add)
        nc.vector.tensor_tensor(phi[:], phi[:], phia[:], ALU.min)
        nc.vector.tensor_scalar(phi[:], phi[:], 0.0, None, ALU.max)
        nc.vector.tensor_tensor(kint[:, 0:9], phi[:, 0:9], phi[:, 9:18], ALU.mult)
        samp_ps = psmall.tile([128, 10], f32, tag="sp")
        nc.tensor.matmul(samp_ps[:], w2dt2[:, li, :], kint[:], start=True, stop=True)
        sampf = small.tile([128, 9], f32, tag="samp")
        nc.vector.tensor_copy(sampf[:], samp_ps[:, 0:9])
        diag = diagp.tile([128, 9, 128], f16, tag="diag")
        for i, t in enumerate(PE_TAPS + DVE_TAPS):
            if i % 2 == 0:
                nc.vector.tensor_scalar(diag[:, t, :], i128[:], sampf[:, t:t + 1],
                                        None, ALU.mult)
            else:
                nc.scalar.activation(diag[:, t, :], i128[:], AF.Copy,
                                     bias=0.0, scale=sampf[:, t:t + 1])

        h_nxt = new_h()
        if li < NL - 1:
            pp_nxt = small.tile([128, NB], f32, tag="pp")
        else:
            pp_nxt = None
        for b in range(NB):
            s0 = 1 + BR * b
            ps = psum.tile([128, BR, HH], f32, tag="cps")
            pe_taps = list(range(9)) if b >= NB - 2 else PE_TAPS
            dve_taps = [] if b >= NB - 2 else DVE_TAPS
            for i, t in enumerate(pe_taps):
                dy, dx = TAPS[t]
                rhs = h_cur[:, s0 + dy:s0 + dy + BR, 1 + dx:1 + dx + HH]
                nc.tensor.matmul(ps[:], diag[:, t, :], rhs,
                                 start=(i == 0), stop=(i == len(pe_taps) - 1))
            for t in dve_taps:
                dy, dx = TAPS[t]
                rhs = h_cur[:, s0 + dy:s0 + dy + BR, 1 + dx:1 + dx + HH]
                nc.vector.scalar_tensor_tensor(ps[:], rhs, sampf[:, t:t + 1],
                                               ps[:], ALU.mult, ALU.add)
            if pp_nxt is not None:
                nc.scalar.activation(h_nxt[:, s0:s0 + BR, 1:HH + 1], ps[:],
                                     AF.Relu, bias=biases[:, 1 + li:2 + li],
                                     scale=1.0, accum_out=pp_nxt[:, b:b + 1])
            else:
                nc.scalar.activation(h_nxt[:, s0:s0 + BR, 1:HH + 1], ps[:],
                                     AF.Relu, bias=biases[:, 1 + li:2 + li],
                                     scale=1.0)
        halo_fix(h_nxt)
        h_cur, pp_cur = h_nxt, pp_nxt

    # ---------------- fused conv2' -> att; selector replicate; out (pipelined) ----------------
    # conv2' band b covers att rows 4b..4b+3 == output rows 8b..8b+7; the
    # final multiply for band b-1 runs while conv2' band b streams.
    for b in range(NB + 1):
        if b < NB:
            s0 = 1 + BR * b
            ps2 = psum.tile([128, BR, HH], f32, tag="cps")
            for t in range(9):
                dy, dx = TAPS[t]
                rhs = h_cur[:, s0 + dy:s0 + dy + BR, 1 + dx:1 + dx + HH]
                nc.tensor.matmul(ps2[0:8, :, :], w2ft[:, t, :], rhs,
                                 start=(t == 0), stop=(t == 8))
            nc.scalar.activation(att8[:, BR * b:BR * b + BR, :], ps2[0:8, :, :],
                                 AF.Sigmoid, bias=b2f8[:], scale=1.0)
        if b > 0:
            bb = b - 1
            ob = outp.tile([128, 2 * BR, H], f32, tag="ob")
            for j in range(4):
                dy, dx = j // 2, j % 2
                pa = psum.tile([128, BR, HH], f32, tag="cps")
                nc.tensor.matmul(pa[:], sel8[:, j, :],
                                 att8[0:8, BR * bb:BR * bb + BR, :],
                                 start=True, stop=True, skip_group_check=True)
                cb = 129 if dx == 0 else 1
                xv = xc[:, 1 + 2 * BR * bb + dy:1 + 2 * BR * bb + dy + 2 * BR - 1:2,
                        cb:cb + HH]
                ov = _ap(ob, dy * H + dx, [ob.ap[0], [2 * H, BR], [2, HH]])
                nc.vector.tensor_tensor(ov, xv, pa[:], ALU.mult)
            y0 = 2 * BR * bb
            nc.sync.dma_start(out=o_d[:, y0:y0 + 2 * BR, :], in_=ob[0:C])
            nc.gpsimd.dma_start(out=o_d[:, 128 + y0:128 + y0 + 2 * BR, :], in_=ob[C:128])


_NC_CACHE = {}


def kernel(**inputs):
    if "nc" not in _NC_CACHE:
        _NC_CACHE["nc"] = build_nc()
    nc = _NC_CACHE["nc"]
    names = ["conv1_w", "conv1_b", "dkc_w", "dkc_b", "dkc_fc_w", "dkc_fc_b",
             "conv2_w", "conv2_b", "conv3_w", "conv3_b"]
    shared = {n: np.ascontiguousarray(np.asarray(inputs[n], dtype=np.float32))
              for n in names}
    x = np.asarray(inputs["x"], dtype=np.float32)
    in_maps = [dict(shared, x=np.ascontiguousarray(x[i])) for i in range(8)]
    r = run_bass_kernel_spmd(nc, in_maps, list(range(8)))
    _NC_CACHE["last_result"] = r
    return np.stack([r.results[i]["out"] for i in range(8)]).astype(np.float32)
